# revision 1
# baseline (speedup 1.0000x reference)
"""Bahdanau attention Trainium2 kernel.

Problem: B=8, T=256, S=256, H=512 (fp32 I/O).
  Ws_q = q @ W_s.T ; Wh_e = e @ W_h.T
  energies[b,t,s] = v . tanh(Ws_q[b,t,:] + Wh_e[b,s,:])   (masked s >= len_b)
  attn = softmax_s(energies); ctx = attn @ e
  out = tanh(concat([ctx, q]) @ W_out.T)

Sharding: sequence-parallel over T — core c handles t in [c*32, (c+1)*32)
for ALL batches. This balances the src_lengths sparsity across cores
(each core's dominant tanh work is 32 * sum_b(len_b) * H instead of
256 * len_b * H for a single batch).

Per-core dataflow (bf16 compute, fp32 accumulation):
  PE   : Ws_q^T [o,t] and Wh_e^T [o,s] projections (o on partitions)
  DVE  : X[o, t, s] = Wh_e^T[o,s] + Ws_q^T[o,t]  (tensor_scalar_add, 4x bf16)
  ACT  : tanh(X) in place, one instruction per (b, o-chunk)
  PE   : energies[t,s] = sum_o v_o X[o,t,s] — M=1 matmuls col-tiled 4-wide
  DMA  : gather PSUM rows {0,32,64,96} -> energies [32t, s]
  DVE/ACT: masked softmax (exp with accum_out for the row sums)
  DMA  : xbar-transpose of weights [32,s] -> [s,32]
  PE   : ctx^T[h,t] = enc^T @ w^T ; out[t,o] = tanh(comb^T.T @ W_out^T)

Measured (8 axon-tunneled trn2 cores, seed-0 src_lengths, sum(len)=1339):
  ~272 us per-core device time (interleaved A/B For_i-loop slope,
  median of 14 pairs; sequential measurements scatter +-50us), rel err
  vs fp32
  reference: 1.3e-2 max, 2.3e-3 L2 (bf16 compute floor). The X tensor is
  built as per-(oc, t-half) half-tiles: Tile tracks dependencies
  per-tile, so separate tiles let each tanh overlap the next half's
  adds (one shared tile with sliced ops falsely serializes). HW notes: DVE tensor_scalar with an
  AP scalar runs at 2x (not the 4x of the immediate form, ~212ns per
  [128,256] bf16 add); ACT tanh matches (F+352)/1.2GHz; PSUM
  accumulation groups must not interleave within a (partition, bank)
  zero-region; DMA cannot read PSUM; single-DMA free-dim->partition
  scatter silently misplaces data (partition->partition is fine).
"""

import functools

import ml_dtypes
import numpy as np

B, T, S, H = 8, 256, 256, 512
NCORES = 8
TC = T // NCORES  # 32 target positions per core
KC = H // 128     # 4 contraction chunks
OC = H // 128     # 4 output-feature chunks

_BF16 = ml_dtypes.bfloat16


def _ceil4(x: int) -> int:
    return max(4, (x + 3) // 4 * 4)


@functools.lru_cache(maxsize=8)
def _build(lens: tuple, loop_n: int | None = None, stages: int = 3):
    """Build + compile the per-core Bass program with per-batch s-extents
    baked in. Same program runs on all 8 cores (inputs differ)."""
    import concourse.mybir as mybir
    import concourse.tile as tile
    from concourse import bacc

    f32 = mybir.dt.float32
    bf16 = mybir.dt.bfloat16
    AF = mybir.ActivationFunctionType
    AX = mybir.AxisListType

    Ls = [_ceil4(l) for l in lens]

    nc = bacc.Bacc("TRN2", target_bir_lowering=False, debug=False)

    # All inputs are host-pre-arranged into SBUF layout [128, free].
    qt_d = nc.dram_tensor("qt", [128, KC, B, TC], bf16, kind="ExternalInput")
    encT_d = nc.dram_tensor("encT", [B, 128, KC, S], bf16, kind="ExternalInput")
    enc_d = nc.dram_tensor("enc", [B, 128, S // 128, H], bf16, kind="ExternalInput")
    wst_d = nc.dram_tensor("wst", [128, KC, H], bf16, kind="ExternalInput")
    wht_d = nc.dram_tensor("wht", [128, KC, H], bf16, kind="ExternalInput")
    v_d = nc.dram_tensor("v", [128, KC], bf16, kind="ExternalInput")
    wot_d = nc.dram_tensor("wot", [128, 2 * KC, H], bf16, kind="ExternalInput")
    out_d = nc.dram_tensor("out", [B, TC, H], f32, kind="ExternalOutput")

    import contextlib

    with tile.TileContext(nc) as tc:
        loop_cm = (
            tc.For_i(
                0, loop_n, 1,
                hint_engines=(
                    mybir.EngineType.PE, mybir.EngineType.DVE,
                    mybir.EngineType.Activation, mybir.EngineType.SP,
                    mybir.EngineType.Pool,
                ),
            )
            if loop_n is not None
            else contextlib.nullcontext()
        )
        with (
            tc.tile_pool(name="const", bufs=1) as constp,
            tc.tile_pool(name="enc", bufs=3) as encp,
            tc.tile_pool(name="es", bufs=2) as esp,
            tc.tile_pool(name="x", bufs=2) as xp,
            tc.tile_pool(name="sm", bufs=3) as smp,
            tc.tile_pool(name="outs", bufs=3) as outp,
            tc.tile_pool(name="psA", bufs=3, space="PSUM") as psA,
            tc.tile_pool(name="psV", bufs=2, space="PSUM") as psV,
            tc.tile_pool(name="psC", bufs=1, space="PSUM") as psC,
            tc.tile_pool(name="psO", bufs=1, space="PSUM") as psO,
            loop_cm,
        ):
            # ---- persistent weights/activations ----
            # DMA order matters for pipeline fill: projQ deps (qt, wst) and
            # projE deps (wht) first; v/wot are not needed until the first
            # tail.
            qt_sb = constp.tile([128, KC, B, TC], bf16)
            nc.sync.dma_start(qt_sb[:], qt_d[:])
            wst = constp.tile([128, KC, H], bf16)
            nc.sync.dma_start(wst[:], wst_d[:])
            wht = constp.tile([128, KC, H], bf16)
            nc.sync.dma_start(wht[:], wht_d[:])
            v_sb = constp.tile([128, KC], bf16)
            nc.sync.dma_start(v_sb[:], v_d[:])
            wot = constp.tile([128, 2 * KC, H], bf16)
            nc.sync.dma_start(wot[:], wot_d[:])

            # ---- Ws_q^T for all (b, t): qs[o-part, oc, b, t] (fp32 scalars) ----
            qs_sb = constp.tile([128, OC, B, TC], f32)
            for oc in range(OC):
                ps = psA.tile([128, B * TC], f32, tag="proj")
                for kc in range(KC):
                    nc.tensor.matmul(
                        ps[:],
                        wst[:, kc, oc * 128 : (oc + 1) * 128],
                        qt_sb[:, kc, :, :],
                        start=(kc == 0),
                        stop=(kc == KC - 1),
                    )
                nc.vector.tensor_copy(
                    qs_sb[:, oc, :, :], ps.rearrange("p (b t) -> p b t", b=B)
                )

            # Software-pipelined emission: engines execute their streams in
            # order, so the tail of batch b (vdot/softmax/ctx/out — gated on
            # long dependency chains) is emitted AFTER the head of batch b+1
            # (proj/adds/tanh). This keeps DVE/ACT streaming without stalls.
            state = {}

            def head(b):
                L = Ls[b]
                # load encoder (both layouts), full-S tiles for clean DMA
                encT_b = encp.tile([128, KC, S], bf16, tag="encT")
                nc.sync.dma_start(encT_b[:], encT_d[b])
                # second HWDGE engine (ACT) -> disjoint queue set; issue cost
                # in the ACT stream is negligible and it has no dependencies
                enc_b = encp.tile([128, S // 128, H], bf16, tag="enc")
                nc.scalar.dma_start(enc_b[:], enc_d[b])

                # Wh_e^T: es[o-part, s] per oc
                es = []
                for oc in range(OC):
                    ps = psA.tile([128, L], f32, tag="proj")
                    for kc in range(KC):
                        nc.tensor.matmul(
                            ps[:],
                            wht[:, kc, oc * 128 : (oc + 1) * 128],
                            encT_b[:, kc, :L],
                            start=(kc == 0),
                            stop=(kc == KC - 1),
                        )
                    e = esp.tile([128, L], bf16, tag=f"es{oc}")
                    nc.vector.tensor_copy(e[:], ps[:])
                    es.append(e)

                # X[o, t, s] = es[o, s] + qs[o, t]; tanh in place.
                # Two half-tiles per (oc): Tile tracks deps coarsely per
                # tile, so separate tiles let the tanh of half 0 overlap
                # the adds of half 1 without false serialization.
                X = []
                for oc in range(OC):
                    halves = []
                    for h2 in range(2):
                        x = xp.tile([128, TC // 2, L], bf16, tag=f"x{oc}h{h2}",
                                    name=f"x{oc}h{h2}")
                        for tl in range(TC // 2 if stages != 5 else 1):
                            t = h2 * (TC // 2) + tl
                            nc.vector.tensor_scalar_add(
                                x[:, tl, :], es[oc][:], qs_sb[:, oc, b, t : t + 1]
                            )
                        if stages not in (4, 5):
                            nc.scalar.activation(x[:], x[:], AF.Tanh)
                        halves.append(x)
                    X.append(halves)
                state[b] = (X, enc_b)

            def tail(b):
                L = Ls[b]
                ln = min(int(lens[b]), S)
                SC = (L + 127) // 128
                L128 = SC * 128
                X, enc_b = state.pop(b)
                if stages == 1:
                    ob = outp.tile([32, 16], f32, tag="ob1")
                    nc.vector.tensor_copy(ob[:], X[0][0][:32, 0, :16])
                    nc.sync.dma_start(out_d[b][:, :16], ob[:])
                    return

                # energies[t, s] = sum_o v_o X[o, t, s]: M=1 matmuls, 16 t's
                # per PSUM tile (4 col groups x 4 bank slots), wide DVE evac,
                # partition->partition DMA gather. Note: accumulation groups
                # sharing a (partition, bank) zero-region must not interleave
                # (start=True marks the whole 2KB bank-row pending-zero);
                # col groups (distinct partitions) may interleave freely.
                energ = smp.tile([32, L], f32, tag="energ")
                for h in range(TC // 8):
                    psq = psV.tile([128, 2, 512 // 2], f32, tag="vdot")
                    for n in range(2):
                        for oc in range(OC):
                            for j in range(4):
                                t = h * 8 + 4 * n + j
                                nc.tensor.matmul(
                                    psq[32 * j : 32 * j + 1, n, :L],
                                    v_sb[:, oc : oc + 1],
                                    X[oc][t // 16][:, t % 16, :],
                                    start=(oc == 0),
                                    stop=(oc == OC - 1),
                                    tile_position=(0, 32 * j),
                                )
                    vscr = smp.tile([128, 2, L], f32, tag="vscr")
                    nc.vector.tensor_copy(vscr[:], psq[:, :, :L])
                    vsr = vscr.rearrange("(g r) n f -> g r n f", r=32)
                    for n in range(2):
                        nc.sync.dma_start(
                            energ[h * 8 + 4 * n : h * 8 + 4 * n + 4, :],
                            vsr[:, 0, n, :],
                        )

                if stages == 2:
                    ob = outp.tile([32, 16], f32, tag="ob1")
                    nc.vector.tensor_copy(ob[:], energ[:, :16])
                    nc.sync.dma_start(out_d[b][:, :16], ob[:])
                    return
                # softmax over s (energies bounded by sum|v| ~ 20: raw exp
                # is safe in fp32/bf16 — skip max-subtract)
                if L > ln:
                    nc.vector.memset(energ[:, ln:L], -1e9)
                w_sb = smp.tile([32, L128], bf16, tag="w")
                if L128 > L:
                    nc.vector.memset(w_sb[:, L:], 0.0)
                sm = smp.tile([32, 1], f32, tag="sm")
                nc.scalar.activation(
                    w_sb[:, :L], energ[:], AF.Exp, accum_out=sm[:]
                )
                rs = smp.tile([32, 1], f32, tag="rs")
                nc.vector.reciprocal(rs[:], sm[:])
                nc.vector.tensor_scalar_mul(w_sb[:, :L], w_sb[:, :L], rs[:])

                # w^T via DMA xbar transpose: [32, L128] -> [L128, 32]
                wT = smp.tile([128, SC, TC], bf16, tag="wT")
                for sc in range(SC):
                    nc.sync.dma_start_transpose(
                        wT[:, sc, :], w_sb[:, sc * 128 : (sc + 1) * 128]
                    )

                # ctx^T[h, t] = sum_s enc[s, h] * w[t, s]; all 4 oc slots in
                # one PSUM bank-row (groups are sequential per slot — legal)
                psc = psC.tile([128, OC, TC], f32, tag="ctx")
                for oc in range(OC):
                    for sc in range(SC):
                        nc.tensor.matmul(
                            psc[:, oc, :],
                            enc_b[:, sc, oc * 128 : (oc + 1) * 128],
                            wT[:, sc, :],
                            start=(sc == 0),
                            stop=(sc == SC - 1),
                        )
                ctxT = outp.tile([128, OC, TC], bf16, tag="ctxT")
                nc.vector.tensor_copy(ctxT[:], psc[:])

                # out[t, o] = tanh(sum_k comb^T[k, t] * W_out[o, k])
                pso = psO.tile([32, H], f32, tag="outp")
                for kc in range(2 * KC):
                    lhsT = (
                        ctxT[:, kc, :]
                        if kc < OC
                        else qt_sb[:, kc - OC, b, :]
                    )
                    nc.tensor.matmul(
                        pso[:],
                        lhsT,
                        wot[:, kc, :],
                        start=(kc == 0),
                        stop=(kc == 2 * KC - 1),
                    )
                ob = outp.tile([32, H], f32, tag="ob")
                nc.scalar.activation(ob[:], pso[:], AF.Tanh)
                nc.sync.dma_start(out_d[b], ob[:])

            # Descending-L order: the pipeline tail drain (last batch's
            # tail with no head to overlap) is paid on the smallest batch.
            order = sorted(range(B), key=lambda b: -Ls[b])
            for i, b in enumerate(order):
                head(b)
                if i > 0:
                    tail(order[i - 1])
            tail(order[-1])

    nc.compile()
    return nc


def _prep_inputs(query, encoder_outputs, src_lengths, W_s, W_h, v, W_out):
    """Host-side: cast to bf16 and pre-arrange into SBUF layouts."""
    q = np.asarray(query, np.float32)
    e = np.asarray(encoder_outputs, np.float32)

    # [128, KC, B, TC] per core: qt[p, kc, b, t] = q[b, c*TC+t, kc*128+p]
    # build once for full T then slice per core.
    qt_full = np.transpose(
        q.reshape(B, T, KC, 128), (2, 3, 0, 1)
    )  # [KC, 128, B, T]
    qt_full = np.ascontiguousarray(np.swapaxes(qt_full, 0, 1)).astype(_BF16)
    # -> [128, KC, B, T]

    # encT[b, p, kc, s] = e[b, s, kc*128+p]
    encT = np.ascontiguousarray(
        np.transpose(e.reshape(B, S, KC, 128), (0, 3, 2, 1))
    ).astype(_BF16)
    # enc[b, p, sc, h] = e[b, sc*128+p, h]
    enc = np.ascontiguousarray(
        np.transpose(e.reshape(B, S // 128, 128, H), (0, 2, 1, 3))
    ).astype(_BF16)

    # wst[p, kc, o] = W_s[o, kc*128+p]
    wst = np.ascontiguousarray(
        np.transpose(np.asarray(W_s, np.float32).reshape(H, KC, 128), (2, 1, 0))
    ).astype(_BF16)
    wht = np.ascontiguousarray(
        np.transpose(np.asarray(W_h, np.float32).reshape(H, KC, 128), (2, 1, 0))
    ).astype(_BF16)
    # v[p, kc] = v[kc*128+p]
    v_pre = np.ascontiguousarray(
        np.asarray(v, np.float32).reshape(KC, 128).T
    ).astype(_BF16)
    # wot[p, kc, o] = W_out[o, kc*128+p]   (k = 2H contraction)
    wot = np.ascontiguousarray(
        np.transpose(np.asarray(W_out, np.float32).reshape(H, 2 * KC, 128), (2, 1, 0))
    ).astype(_BF16)

    lens = tuple(int(x) for x in np.asarray(src_lengths).reshape(-1))
    return qt_full, encT, enc, wst, wht, v_pre, wot, lens


def kernel(query, encoder_outputs, src_lengths, W_s, W_h, v, W_out):
    from concourse import bass_utils

    qt_full, encT, enc, wst, wht, v_pre, wot, lens = _prep_inputs(
        query, encoder_outputs, src_lengths, W_s, W_h, v, W_out
    )
    nc = _build(lens)

    in_maps = []
    for c in range(NCORES):
        qt_c = np.ascontiguousarray(
            qt_full[:, :, :, c * TC : (c + 1) * TC]
        )
        in_maps.append(
            {
                "qt": qt_c,
                "encT": encT,
                "enc": enc,
                "wst": wst,
                "wht": wht,
                "v": v_pre,
                "wot": wot,
            }
        )

    res = bass_utils.run_bass_kernel_spmd(nc, in_maps, core_ids=list(range(NCORES)))

    out = np.empty((B, T, H), np.float32)
    for c in range(NCORES):
        out[:, c * TC : (c + 1) * TC, :] = res.results[c]["out"]
    return out



# revision 2
# speedup vs baseline: 1.0865x; 1.0865x over previous
"""Bahdanau attention Trainium2 kernel.

Problem: B=8, T=256, S=256, H=512 (fp32 I/O).
  Ws_q = q @ W_s.T ; Wh_e = e @ W_h.T
  energies[b,t,s] = v . tanh(Ws_q[b,t,:] + Wh_e[b,s,:])   (masked s >= len_b)
  attn = softmax_s(energies); ctx = attn @ e
  out = tanh(concat([ctx, q]) @ W_out.T)

Sharding: sequence-parallel over T — core c handles t in [c*32, (c+1)*32)
for ALL batches, balancing src_lengths sparsity across cores.

Per-core dataflow (bf16 compute, fp32 accumulation):
  PE   : Ws_q^T [o,t] and Wh_e^T [o,s] projections (o on partitions)
  DVE  : X[o, t, s] = es[o,s] + qs[o,t] for oc 0..2 (per-t tensor_scalar,
         the fastest HW path for this broadcast: measured ~c0+0.7ns/elem;
         DVE tensor_tensor with a stride-0-innermost operand runs ~1.3ns/elem)
  Pool : oc 3's adds as ONE broadcast tensor_tensor per batch (GPSIMD
         handles broadcast APs at its usual ~1.6ns/elem; otherwise idle)
  ACT  : tanh(X) in place, one instruction per (b, oc) [F = 32*L]
         (ACT measured ~145ns fixed + ~1.0ns/free-elem; ACT is the
         critical engine at ~24us/batch — everything else hides under it)
  PE   : energies[t,s] = sum_o v_o X[o,t,s] — M=1 matmuls col-tiled 4-wide
  DMA  : gather PSUM rows {0,32,64,96} -> energies [32t, s]
  DVE/ACT: masked softmax (exp with accum_out for the row sums)
  DMA  : xbar-transpose of weights [32,s] -> [s,32]
  PE   : ctx^T[h,t] = enc^T @ w^T ; out[t,o] = tanh(comb^T.T @ W_out^T)

HW notes: PSUM accumulation groups must not interleave within a
(partition, bank) zero-region; DMA cannot read PSUM; single-DMA
free-dim->partition scatter silently misplaces data.
"""

import functools

import ml_dtypes
import numpy as np

B, T, S, H = 8, 256, 256, 512
NCORES = 8
TC = T // NCORES  # 32 target positions per core
KC = H // 128     # 4 contraction chunks
OC = H // 128     # 4 output-feature chunks
POOL_OC = 3       # oc chunk handled by the GPSIMD (Pool) engine

_BF16 = ml_dtypes.bfloat16


def _ceil4(x: int) -> int:
    return max(4, (x + 3) // 4 * 4)


@functools.lru_cache(maxsize=8)
def _build(lens: tuple, loop_n: int | None = None):
    """Build + compile the per-core Bass program with per-batch s-extents
    baked in. Same program runs on all 8 cores (inputs differ)."""
    import concourse.mybir as mybir
    import concourse.tile as tile
    from concourse import bacc

    f32 = mybir.dt.float32
    bf16 = mybir.dt.bfloat16
    AF = mybir.ActivationFunctionType
    ALU = mybir.AluOpType

    Ls = [_ceil4(l) for l in lens]

    nc = bacc.Bacc("TRN2", target_bir_lowering=False, debug=False)

    # All inputs are host-pre-arranged into SBUF layout [128, free].
    qt_d = nc.dram_tensor("qt", [128, KC, B, TC], bf16, kind="ExternalInput")
    encT_d = nc.dram_tensor("encT", [B, 128, KC, S], bf16, kind="ExternalInput")
    enc_d = nc.dram_tensor("enc", [B, 128, S // 128, H], bf16, kind="ExternalInput")
    wst_d = nc.dram_tensor("wst", [128, KC, H], bf16, kind="ExternalInput")
    wht_d = nc.dram_tensor("wht", [128, KC, H], bf16, kind="ExternalInput")
    v_d = nc.dram_tensor("v", [128, KC], bf16, kind="ExternalInput")
    wot_d = nc.dram_tensor("wot", [128, 2 * KC, H], bf16, kind="ExternalInput")
    out_d = nc.dram_tensor("out", [B, TC, H], f32, kind="ExternalOutput")

    import contextlib

    with tile.TileContext(nc) as tc:
        loop_cm = (
            tc.For_i(
                0, loop_n, 1,
                hint_engines=(
                    mybir.EngineType.PE, mybir.EngineType.DVE,
                    mybir.EngineType.Activation, mybir.EngineType.SP,
                    mybir.EngineType.Pool,
                ),
            )
            if loop_n is not None
            else contextlib.nullcontext()
        )
        with (
            tc.tile_pool(name="const", bufs=1) as constp,
            tc.tile_pool(name="enc", bufs=3) as encp,
            tc.tile_pool(name="es", bufs=2) as esp,
            tc.tile_pool(name="x", bufs=2) as xp,
            tc.tile_pool(name="sm", bufs=3) as smp,
            tc.tile_pool(name="outs", bufs=3) as outp,
            tc.tile_pool(name="psA", bufs=3, space="PSUM") as psA,
            tc.tile_pool(name="psV", bufs=2, space="PSUM") as psV,
            tc.tile_pool(name="psC", bufs=1, space="PSUM") as psC,
            tc.tile_pool(name="psO", bufs=1, space="PSUM") as psO,
            loop_cm,
        ):
            # ---- persistent weights/activations ----
            # DMA order matters for pipeline fill: projQ deps (qt, wst) and
            # projE deps (wht) first; v/wot are not needed until the first
            # tail.
            qt_sb = constp.tile([128, KC, B, TC], bf16)
            nc.sync.dma_start(qt_sb[:], qt_d[:])
            wst = constp.tile([128, KC, H], bf16)
            nc.sync.dma_start(wst[:], wst_d[:])
            wht = constp.tile([128, KC, H], bf16)
            nc.sync.dma_start(wht[:], wht_d[:])
            v_sb = constp.tile([128, KC], bf16)
            nc.sync.dma_start(v_sb[:], v_d[:])
            wot = constp.tile([128, 2 * KC, H], bf16)
            nc.sync.dma_start(wot[:], wot_d[:])

            # ---- Ws_q^T for all (b, t): qs[o-part, oc, b, t] ----
            # f32 copy feeds DVE tensor_scalar (scalar must be f32);
            # bf16 copy feeds Pool's broadcast tensor_tensor.
            qs_sb = constp.tile([128, OC, B, TC], f32)
            qb_sb = constp.tile([128, OC, B, TC], bf16)
            for oc in range(OC):
                ps = psA.tile([128, B * TC], f32, tag="proj")
                for kc in range(KC):
                    nc.tensor.matmul(
                        ps[:],
                        wst[:, kc, oc * 128 : (oc + 1) * 128],
                        qt_sb[:, kc, :, :],
                        start=(kc == 0),
                        stop=(kc == KC - 1),
                    )
                nc.vector.tensor_copy(
                    qs_sb[:, oc, :, :], ps.rearrange("p (b t) -> p b t", b=B)
                )
                nc.vector.tensor_copy(
                    qb_sb[:, oc, :, :], ps.rearrange("p (b t) -> p b t", b=B)
                )

            # Software-pipelined emission: engines execute their streams in
            # order, so the tail of batch b (vdot/softmax/ctx/out — gated on
            # long dependency chains) is emitted AFTER the head of batch b+1
            # (proj/adds/tanh). This keeps DVE/ACT streaming without stalls.
            state = {}

            def head(b):
                L = Ls[b]
                # load encoder (both layouts), full-S tiles for clean DMA
                encT_b = encp.tile([128, KC, S], bf16, tag="encT")
                nc.sync.dma_start(encT_b[:], encT_d[b])
                # second HWDGE engine (ACT) -> disjoint queue set; issue cost
                # in the ACT stream is negligible and it has no dependencies
                enc_b = encp.tile([128, S // 128, H], bf16, tag="enc")
                nc.scalar.dma_start(enc_b[:], enc_d[b])

                # Wh_e^T: es[o-part, s] per oc
                es = []
                for oc in range(OC):
                    ps = psA.tile([128, L], f32, tag="proj")
                    for kc in range(KC):
                        nc.tensor.matmul(
                            ps[:],
                            wht[:, kc, oc * 128 : (oc + 1) * 128],
                            encT_b[:, kc, :L],
                            start=(kc == 0),
                            stop=(kc == KC - 1),
                        )
                    e = esp.tile([128, L], bf16, tag=f"es{oc}")
                    nc.vector.tensor_copy(e[:], ps[:])
                    es.append(e)

                # X[o, t, s] = es[o, s] + qs[o, t]; tanh in place.
                # One tile per oc so tanh(oc) overlaps the adds of oc+1.
                # Pool's oc is emitted first: its single broadcast add runs
                # concurrently with DVE's per-t adds for the other three.
                X = [None] * OC
                x3 = xp.tile([128, TC, L], bf16, tag=f"x{POOL_OC}",
                             name=f"x{POOL_OC}")
                nc.gpsimd.tensor_tensor(
                    x3[:],
                    es[POOL_OC][:, None, :].broadcast_to([128, TC, L]),
                    qb_sb[:, POOL_OC, b, :, None].broadcast_to([128, TC, L]),
                    ALU.add,
                )
                X[POOL_OC] = x3
                for oc in range(OC):
                    if oc == POOL_OC:
                        continue
                    x = xp.tile([128, TC, L], bf16, tag=f"x{oc}", name=f"x{oc}")
                    for t in range(TC):
                        nc.vector.tensor_scalar_add(
                            x[:, t, :], es[oc][:], qs_sb[:, oc, b, t : t + 1]
                        )
                    X[oc] = x
                for oc in range(OC):
                    if oc != POOL_OC:
                        nc.scalar.activation(X[oc][:], X[oc][:], AF.Tanh)
                nc.scalar.activation(x3[:], x3[:], AF.Tanh)
                state[b] = (X, enc_b)

            def tail(b):
                L = Ls[b]
                ln = min(int(lens[b]), S)
                SC = (L + 127) // 128
                L128 = SC * 128
                X, enc_b = state.pop(b)

                # energies[t, s] = sum_o v_o X[o, t, s]: M=1 matmuls, 16 t's
                # per PSUM tile (4 col groups x 4 bank slots), wide DVE evac,
                # partition->partition DMA gather. Note: accumulation groups
                # sharing a (partition, bank) zero-region must not interleave
                # (start=True marks the whole 2KB bank-row pending-zero);
                # col groups (distinct partitions) may interleave freely.
                energ = smp.tile([32, L], f32, tag="energ")
                for h in range(TC // 8):
                    psq = psV.tile([128, 2, 512 // 2], f32, tag="vdot")
                    for n in range(2):
                        for oc in range(OC):
                            for j in range(4):
                                t = h * 8 + 4 * n + j
                                nc.tensor.matmul(
                                    psq[32 * j : 32 * j + 1, n, :L],
                                    v_sb[:, oc : oc + 1],
                                    X[oc][:, t, :],
                                    start=(oc == 0),
                                    stop=(oc == OC - 1),
                                    tile_position=(0, 32 * j),
                                )
                    vscr = smp.tile([128, 2, L], f32, tag="vscr")
                    nc.vector.tensor_copy(vscr[:], psq[:, :, :L])
                    vsr = vscr.rearrange("(g r) n f -> g r n f", r=32)
                    for n in range(2):
                        nc.sync.dma_start(
                            energ[h * 8 + 4 * n : h * 8 + 4 * n + 4, :],
                            vsr[:, 0, n, :],
                        )

                # softmax over s (energies bounded by sum|v| ~ 20: raw exp
                # is safe in fp32/bf16 — skip max-subtract)
                if L > ln:
                    nc.vector.memset(energ[:, ln:L], -1e9)
                w_sb = smp.tile([32, L128], bf16, tag="w")
                if L128 > L:
                    nc.gpsimd.memset(w_sb[:, L:], 0.0)
                sm = smp.tile([32, 1], f32, tag="sm")
                nc.scalar.activation(
                    w_sb[:, :L], energ[:], AF.Exp, accum_out=sm[:]
                )
                rs = smp.tile([32, 1], f32, tag="rs")
                nc.vector.reciprocal(rs[:], sm[:])
                nc.vector.tensor_scalar_mul(w_sb[:, :L], w_sb[:, :L], rs[:])

                # w^T via DMA xbar transpose: [32, L128] -> [L128, 32]
                wT = smp.tile([128, SC, TC], bf16, tag="wT")
                for sc in range(SC):
                    nc.sync.dma_start_transpose(
                        wT[:, sc, :], w_sb[:, sc * 128 : (sc + 1) * 128]
                    )

                # ctx^T[h, t] = sum_s enc[s, h] * w[t, s]; all 4 oc slots in
                # one PSUM bank-row (groups are sequential per slot — legal)
                psc = psC.tile([128, OC, TC], f32, tag="ctx")
                for oc in range(OC):
                    for sc in range(SC):
                        nc.tensor.matmul(
                            psc[:, oc, :],
                            enc_b[:, sc, oc * 128 : (oc + 1) * 128],
                            wT[:, sc, :],
                            start=(sc == 0),
                            stop=(sc == SC - 1),
                        )
                ctxT = outp.tile([128, OC, TC], bf16, tag="ctxT")
                nc.vector.tensor_copy(ctxT[:], psc[:])

                # out[t, o] = tanh(sum_k comb^T[k, t] * W_out[o, k])
                pso = psO.tile([32, H], f32, tag="outp")
                for kc in range(2 * KC):
                    lhsT = (
                        ctxT[:, kc, :]
                        if kc < OC
                        else qt_sb[:, kc - OC, b, :]
                    )
                    nc.tensor.matmul(
                        pso[:],
                        lhsT,
                        wot[:, kc, :],
                        start=(kc == 0),
                        stop=(kc == 2 * KC - 1),
                    )
                ob = outp.tile([32, H], f32, tag="ob")
                nc.scalar.activation(ob[:], pso[:], AF.Tanh)
                nc.sync.dma_start(out_d[b], ob[:])

            # Descending-L order: the pipeline tail drain (last batch's
            # tail with no head to overlap) is paid on the smallest batch.
            order = sorted(range(B), key=lambda b: -Ls[b])
            for i, b in enumerate(order):
                head(b)
                if i > 0:
                    tail(order[i - 1])
            tail(order[-1])

    nc.compile()
    return nc


def _prep_inputs(query, encoder_outputs, src_lengths, W_s, W_h, v, W_out):
    """Host-side: cast to bf16 and pre-arrange into SBUF layouts."""
    q = np.asarray(query, np.float32)
    e = np.asarray(encoder_outputs, np.float32)

    # [128, KC, B, TC] per core: qt[p, kc, b, t] = q[b, c*TC+t, kc*128+p]
    # build once for full T then slice per core.
    qt_full = np.transpose(
        q.reshape(B, T, KC, 128), (2, 3, 0, 1)
    )  # [KC, 128, B, T]
    qt_full = np.ascontiguousarray(np.swapaxes(qt_full, 0, 1)).astype(_BF16)
    # -> [128, KC, B, T]

    # encT[b, p, kc, s] = e[b, s, kc*128+p]
    encT = np.ascontiguousarray(
        np.transpose(e.reshape(B, S, KC, 128), (0, 3, 2, 1))
    ).astype(_BF16)
    # enc[b, p, sc, h] = e[b, sc*128+p, h]
    enc = np.ascontiguousarray(
        np.transpose(e.reshape(B, S // 128, 128, H), (0, 2, 1, 3))
    ).astype(_BF16)

    # wst[p, kc, o] = W_s[o, kc*128+p]
    wst = np.ascontiguousarray(
        np.transpose(np.asarray(W_s, np.float32).reshape(H, KC, 128), (2, 1, 0))
    ).astype(_BF16)
    wht = np.ascontiguousarray(
        np.transpose(np.asarray(W_h, np.float32).reshape(H, KC, 128), (2, 1, 0))
    ).astype(_BF16)
    # v[p, kc] = v[kc*128+p]
    v_pre = np.ascontiguousarray(
        np.asarray(v, np.float32).reshape(KC, 128).T
    ).astype(_BF16)
    # wot[p, kc, o] = W_out[o, kc*128+p]   (k = 2H contraction)
    wot = np.ascontiguousarray(
        np.transpose(np.asarray(W_out, np.float32).reshape(H, 2 * KC, 128), (2, 1, 0))
    ).astype(_BF16)

    lens = tuple(int(x) for x in np.asarray(src_lengths).reshape(-1))
    return qt_full, encT, enc, wst, wht, v_pre, wot, lens


def kernel(query, encoder_outputs, src_lengths, W_s, W_h, v, W_out):
    from concourse import bass_utils

    qt_full, encT, enc, wst, wht, v_pre, wot, lens = _prep_inputs(
        query, encoder_outputs, src_lengths, W_s, W_h, v, W_out
    )
    nc = _build(lens)

    in_maps = []
    for c in range(NCORES):
        qt_c = np.ascontiguousarray(
            qt_full[:, :, :, c * TC : (c + 1) * TC]
        )
        in_maps.append(
            {
                "qt": qt_c,
                "encT": encT,
                "enc": enc,
                "wst": wst,
                "wht": wht,
                "v": v_pre,
                "wot": wot,
            }
        )

    res = bass_utils.run_bass_kernel_spmd(nc, in_maps, core_ids=list(range(NCORES)))

    out = np.empty((B, T, H), np.float32)
    for c in range(NCORES):
        out[:, c * TC : (c + 1) * TC, :] = res.results[c]["out"]
    return out


# revision 35
# speedup vs baseline: 3.4900x; 3.2121x over previous
"""Bahdanau attention Trainium2 kernel.

Problem: B=8, T=256, S=256, H=512 (fp32 I/O).
  Ws_q = q @ W_s.T ; Wh_e = e @ W_h.T
  energies[b,t,s] = v . tanh(Ws_q[b,t,:] + Wh_e[b,s,:])   (masked s >= len_b)
  attn = softmax_s(energies); ctx = attn @ e
  out = tanh(concat([ctx, q]) @ W_out.T)

Sharding: sequence-parallel over T — core c handles t in [c*32, (c+1)*32)
for ALL batches, balancing src_lengths sparsity across cores.

Per-core dataflow (bf16 compute, fp32 accumulation):
  PE   : Ws_q^T [o,t] and Wh_e^T [o,s] projections (o on partitions)
  DVE  : X[o, t, s] = es[o,s] + qs[o,t] (per-t tensor_scalar into
         per-(oc, t-half) half-tiles so each tanh overlaps the next
         half's adds; one shared tile with sliced ops falsely serializes)
  ACT  : tanh(X) in place, one instruction per (b, oc, half) [F = 16*L]
  PE   : energies[t,s] = sum_o v_o X[o,t,s] — M=1 matmuls col-tiled 4-wide
  DMA  : gather PSUM rows {0,32,64,96} -> energies [32t, s]
  DVE/ACT: masked softmax (exp over the first len_b cols + zeroed weight
         tail, with accum_out for the row sums)
  DMA  : xbar-transpose of weights [32,s] -> [s,32]
  PE   : ctx^T[h,t] = enc^T @ w^T ; out[t,o] = tanh(comb^T.T @ W_out^T)

The kernel is DVE-bound: the 1024 per-t adds cost ~200ns each in
context (~205us), with ACT at ~195us and PE at ~135us. Measured HW
instruction rates (big-loop slope microbenches): DVE tensor_scalar
~165-260ns for F=128-512 (mostly fixed cost); ACT ~145ns + ~1.0ns/
free-elem; GPSIMD ~230ns + 1.6ns/elem.

Rebalancing attempts that measurably HURT on HW (interleaved A/B, all
reverted — engine streams are in-order, and cross-engine sync costs
dominate fine-grained offload):
  - GPSIMD broadcast adds for any slice of X (whole oc, halves, or 2-4
    t's per half): +15-50us. GPSIMD sem waits/completion are ~us-scale.
  - Fused bias-tanh on ACT for k of 16 t's (tanh(es + qs_t), F=L):
    monotonically worse with k (~+300ns/instr in-context, not ~145).
  - es/vscr PSUM evacs on ACT (count-neutral): +13/+36us — the copies
    queue behind ~6us tanh instructions and starve DVE/exp.
  - GPSIMD cannot read PSUM at all (BIR verifier rejects).

HW notes: PSUM accumulation groups must not interleave within a
(partition, bank) zero-region; DMA cannot read PSUM; single-DMA
free-dim->partition scatter silently misplaces data; energ-style
[4, 256] partition-gather DMA ~500ns, [32,128] xbar transpose ~1.3us.
"""

import functools

import ml_dtypes
import numpy as np

B, T, S, H = 8, 256, 256, 512
NCORES = 8
TC = T // NCORES  # 32 target positions per core
KC = H // 128     # 4 contraction chunks
OC = H // 128     # 4 output-feature chunks

_BF16 = ml_dtypes.bfloat16


def _ceil4(x: int) -> int:
    return max(4, (x + 3) // 4 * 4)


@functools.lru_cache(maxsize=8)
def _build(lens: tuple, loop_n: int | None = None, stages: int = 3):
    """Build + compile the per-core Bass program with per-batch s-extents
    baked in. Same program runs on all 8 cores (inputs differ)."""
    import concourse.mybir as mybir
    import concourse.tile as tile
    from concourse import bacc

    f32 = mybir.dt.float32
    bf16 = mybir.dt.bfloat16
    AF = mybir.ActivationFunctionType

    Ls = [_ceil4(l) for l in lens]

    nc = bacc.Bacc("TRN2", target_bir_lowering=False, debug=False)

    # All inputs are host-pre-arranged into SBUF layout [128, free].
    qt_d = nc.dram_tensor("qt", [128, KC, B, TC], bf16, kind="ExternalInput")
    encT_d = nc.dram_tensor("encT", [B, 128, KC, S], bf16, kind="ExternalInput")
    enc_d = nc.dram_tensor("enc", [B, 128, S // 128, H], bf16, kind="ExternalInput")
    wst_d = nc.dram_tensor("wst", [128, KC, H], bf16, kind="ExternalInput")
    wht_d = nc.dram_tensor("wht", [128, KC, H], bf16, kind="ExternalInput")
    v_d = nc.dram_tensor("v", [128, KC], bf16, kind="ExternalInput")
    wot_d = nc.dram_tensor("wot", [128, 2 * KC, H], bf16, kind="ExternalInput")
    out_d = nc.dram_tensor("out", [B, TC, H], f32, kind="ExternalOutput")

    import contextlib

    with tile.TileContext(nc) as tc:
        loop_cm = (
            tc.For_i(
                0, loop_n, 1,
                hint_engines=(
                    mybir.EngineType.PE, mybir.EngineType.DVE,
                    mybir.EngineType.Activation, mybir.EngineType.SP,
                    mybir.EngineType.Pool,
                ),
            )
            if loop_n is not None
            else contextlib.nullcontext()
        )
        with (
            tc.tile_pool(name="const", bufs=1) as constp,
            tc.tile_pool(name="enc", bufs=3) as encp,
            tc.tile_pool(name="es", bufs=2) as esp,
            tc.tile_pool(name="x", bufs=2) as xp,
            tc.tile_pool(name="sm", bufs=3) as smp,
            tc.tile_pool(name="outs", bufs=3) as outp,
            tc.tile_pool(name="psA", bufs=3, space="PSUM") as psA,
            tc.tile_pool(name="psV", bufs=2, space="PSUM") as psV,
            tc.tile_pool(name="psC", bufs=1, space="PSUM") as psC,
            tc.tile_pool(name="psO", bufs=1, space="PSUM") as psO,
            loop_cm,
        ):
            # ---- persistent weights/activations ----
            # DMA order matters for pipeline fill: projQ deps (qt, wst) and
            # projE deps (wht) first; v/wot are not needed until the first
            # tail.
            # Two HWDGE queues in parallel: projQ deps (qt, wst) on the SP
            # queue; projE dep (wht) + late consts (v, wot) on the ACT
            # queue, so the first Wh_e projection isn't serialized behind
            # the full weight load.
            qt_sb = constp.tile([128, KC, B, TC], bf16)
            nc.sync.dma_start(qt_sb[:], qt_d[:])
            wst = constp.tile([128, KC, H], bf16)
            nc.sync.dma_start(wst[:], wst_d[:])
            wht = constp.tile([128, KC, H], bf16)
            nc.scalar.dma_start(wht[:], wht_d[:])
            v_sb = constp.tile([128, KC], bf16)
            nc.scalar.dma_start(v_sb[:], v_d[:])
            wot = constp.tile([128, 2 * KC, H], bf16)
            nc.scalar.dma_start(wot[:], wot_d[:])

            # ---- Ws_q^T for all (b, t): qs[o-part, oc, b, t] ----
            # f32 copy feeds DVE tensor_scalar (scalar must be f32);
            # bf16 copy feeds Pool's broadcast tensor_tensor.
            qs_sb = constp.tile([128, OC, B, TC], f32)
            for oc in range(OC):
                ps = psA.tile([128, B * TC], f32, tag="proj")
                for kc in range(KC):
                    nc.tensor.matmul(
                        ps[:],
                        wst[:, kc, oc * 128 : (oc + 1) * 128],
                        qt_sb[:, kc, :, :],
                        start=(kc == 0),
                        stop=(kc == KC - 1),
                    )
                nc.scalar.copy(
                    qs_sb[:, oc, :, :], ps.rearrange("p (b t) -> p b t", b=B)
                )

            # Software-pipelined emission: engines execute their streams in
            # order, so the tail of batch b (vdot/softmax/ctx/out — gated on
            # long dependency chains) is emitted AFTER the head of batch b+1
            # (proj/adds/tanh). This keeps DVE/ACT streaming without stalls.
            state = {}

            def head(b):
                L = Ls[b]
                # load encoder (both layouts), full-S tiles for clean DMA
                encT_b = encp.tile([128, KC, S], bf16, tag="encT")
                nc.sync.dma_start(encT_b[:], encT_d[b])
                # second HWDGE engine (ACT) -> disjoint queue set; issue cost
                # in the ACT stream is negligible and it has no dependencies
                enc_b = encp.tile([128, S // 128, H], bf16, tag="enc")
                nc.scalar.dma_start(enc_b[:], enc_d[b])

                # Wh_e^T: es[o-part, s] per oc
                es = []
                for oc in range(OC):
                    ps = psA.tile([128, L], f32, tag="proj")
                    for kc in range(KC):
                        nc.tensor.matmul(
                            ps[:],
                            wht[:, kc, oc * 128 : (oc + 1) * 128],
                            encT_b[:, kc, :L],
                            start=(kc == 0),
                            stop=(kc == KC - 1),
                        )
                    e = esp.tile([128, L], bf16, tag=f"es{oc}")
                    nc.vector.tensor_copy(e[:], ps[:])
                    es.append(e)

                # X[o, t, s] = es[o, s] + qs[o, t]; tanh in place.
                # Two half-tiles per (oc): Tile tracks deps coarsely per
                # tile, so separate tiles let the tanh of half 0 overlap
                # the adds of half 1 without false serialization.
                HT = TC // 2
                X = []
                for oc in range(OC):
                    halves = []
                    for h2 in range(2):
                        x = xp.tile([128, HT, L], bf16, tag=f"x{oc}h{h2}",
                                    name=f"x{oc}h{h2}")
                        for tl in range(HT):
                            t = h2 * HT + tl
                            nc.vector.tensor_scalar_add(
                                x[:, tl, :], es[oc][:], qs_sb[:, oc, b, t : t + 1]
                            )
                        if stages != 4:
                            nc.scalar.activation(x[:], x[:], AF.Tanh)
                        halves.append(x)
                    X.append(halves)
                state[b] = (X, enc_b)

            def tail(b):
                L = Ls[b]
                ln = min(int(lens[b]), S)
                SC = (L + 127) // 128
                L128 = SC * 128
                X, enc_b = state.pop(b)
                if stages == 1:
                    ob = outp.tile([32, 16], f32, tag="ob1")
                    nc.vector.tensor_copy(ob[:], X[0][0][:32, 0, :16])
                    nc.sync.dma_start(out_d[b][:, :16], ob[:])
                    return

                # energies[t, s] = sum_o v_o X[o, t, s]: M=1 matmuls, 16 t's
                # per PSUM tile (4 col groups x 4 bank slots), wide DVE evac,
                # partition->partition DMA gather. Note: accumulation groups
                # sharing a (partition, bank) zero-region must not interleave
                # (start=True marks the whole 2KB bank-row pending-zero);
                # col groups (distinct partitions) may interleave freely.
                energ = smp.tile([32, L], f32, tag="energ")
                for h in range(TC // 8):
                    psq = psV.tile([128, 2, 512 // 2], f32, tag="vdot")
                    for n in range(2):
                        for oc in range(OC):
                            for j in range(4):
                                t = h * 8 + 4 * n + j
                                nc.tensor.matmul(
                                    psq[32 * j : 32 * j + 1, n, :L],
                                    v_sb[:, oc : oc + 1],
                                    X[oc][t // 16][:, t % 16, :],
                                    start=(oc == 0),
                                    stop=(oc == OC - 1),
                                    tile_position=(0, 32 * j),
                                )
                    vscr = smp.tile([128, 2, L], f32, tag="vscr")
                    nc.vector.tensor_copy(vscr[:], psq[:, :, :L])
                    vsr = vscr.rearrange("(g r) n f -> g r n f", r=32)
                    for n in range(2):
                        nc.sync.dma_start(
                            energ[h * 8 + 4 * n : h * 8 + 4 * n + 4, :],
                            vsr[:, 0, n, :],
                        )

                if stages == 2:
                    ob = outp.tile([32, 16], f32, tag="ob1")
                    nc.vector.tensor_copy(ob[:], energ[:, :16])
                    nc.sync.dma_start(out_d[b][:, :16], ob[:])
                    return
                # softmax over s (energies bounded by sum|v| ~ 20: raw exp
                # is safe in fp32/bf16 — skip max-subtract). The s >= len_b
                # mask is applied by exp-ing only the first ln columns and
                # zeroing the weight tail, rather than a -inf energy fill.
                w_sb = smp.tile([32, L128], bf16, tag="w")
                if L128 > ln:
                    nc.vector.memset(w_sb[:, ln:], 0.0)
                sm = smp.tile([32, 1], f32, tag="sm")
                nc.scalar.activation(
                    w_sb[:, :ln], energ[:, :ln], AF.Exp, accum_out=sm[:]
                )
                rs = smp.tile([32, 1], f32, tag="rs")
                nc.vector.reciprocal(rs[:], sm[:])
                nc.vector.tensor_scalar_mul(w_sb[:, :ln], w_sb[:, :ln], rs[:])

                # w^T via DMA xbar transpose: [32, L128] -> [L128, 32]
                wT = smp.tile([128, SC, TC], bf16, tag="wT")
                for sc in range(SC):
                    nc.sync.dma_start_transpose(
                        wT[:, sc, :], w_sb[:, sc * 128 : (sc + 1) * 128]
                    )

                # ctx^T[h, t] = sum_s enc[s, h] * w[t, s]; all 4 oc slots in
                # one PSUM bank-row (groups are sequential per slot — legal)
                psc = psC.tile([128, OC, TC], f32, tag="ctx")
                for oc in range(OC):
                    for sc in range(SC):
                        nc.tensor.matmul(
                            psc[:, oc, :],
                            enc_b[:, sc, oc * 128 : (oc + 1) * 128],
                            wT[:, sc, :],
                            start=(sc == 0),
                            stop=(sc == SC - 1),
                        )
                ctxT = outp.tile([128, OC, TC], bf16, tag="ctxT")
                nc.vector.tensor_copy(ctxT[:], psc[:])

                # out[t, o] = tanh(sum_k comb^T[k, t] * W_out[o, k])
                pso = psO.tile([32, H], f32, tag="outp")
                for kc in range(2 * KC):
                    lhsT = (
                        ctxT[:, kc, :]
                        if kc < OC
                        else qt_sb[:, kc - OC, b, :]
                    )
                    nc.tensor.matmul(
                        pso[:],
                        lhsT,
                        wot[:, kc, :],
                        start=(kc == 0),
                        stop=(kc == 2 * KC - 1),
                    )
                ob = outp.tile([32, H], f32, tag="ob")
                nc.scalar.activation(ob[:], pso[:], AF.Tanh)
                nc.sync.dma_start(out_d[b], ob[:])

            # Descending-L order: the pipeline tail drain (last batch's
            # tail with no head to overlap) is paid on the smallest batch.
            order = sorted(range(B), key=lambda b: -Ls[b])
            for i, b in enumerate(order):
                head(b)
                if i > 0:
                    tail(order[i - 1])
            tail(order[-1])

    nc.compile()
    return nc


def _prep_inputs(query, encoder_outputs, src_lengths, W_s, W_h, v, W_out):
    """Host-side: cast to bf16 and pre-arrange into SBUF layouts."""
    q = np.asarray(query, np.float32)
    e = np.asarray(encoder_outputs, np.float32)

    # [128, KC, B, TC] per core: qt[p, kc, b, t] = q[b, c*TC+t, kc*128+p]
    # build once for full T then slice per core.
    qt_full = np.transpose(
        q.reshape(B, T, KC, 128), (2, 3, 0, 1)
    )  # [KC, 128, B, T]
    qt_full = np.ascontiguousarray(np.swapaxes(qt_full, 0, 1)).astype(_BF16)
    # -> [128, KC, B, T]

    # encT[b, p, kc, s] = e[b, s, kc*128+p]
    encT = np.ascontiguousarray(
        np.transpose(e.reshape(B, S, KC, 128), (0, 3, 2, 1))
    ).astype(_BF16)
    # enc[b, p, sc, h] = e[b, sc*128+p, h]
    enc = np.ascontiguousarray(
        np.transpose(e.reshape(B, S // 128, 128, H), (0, 2, 1, 3))
    ).astype(_BF16)

    # wst[p, kc, o] = W_s[o, kc*128+p]
    wst = np.ascontiguousarray(
        np.transpose(np.asarray(W_s, np.float32).reshape(H, KC, 128), (2, 1, 0))
    ).astype(_BF16)
    wht = np.ascontiguousarray(
        np.transpose(np.asarray(W_h, np.float32).reshape(H, KC, 128), (2, 1, 0))
    ).astype(_BF16)
    # v[p, kc] = v[kc*128+p]
    v_pre = np.ascontiguousarray(
        np.asarray(v, np.float32).reshape(KC, 128).T
    ).astype(_BF16)
    # wot[p, kc, o] = W_out[o, kc*128+p]   (k = 2H contraction)
    wot = np.ascontiguousarray(
        np.transpose(np.asarray(W_out, np.float32).reshape(H, 2 * KC, 128), (2, 1, 0))
    ).astype(_BF16)

    lens = tuple(int(x) for x in np.asarray(src_lengths).reshape(-1))
    return qt_full, encT, enc, wst, wht, v_pre, wot, lens


def kernel(query, encoder_outputs, src_lengths, W_s, W_h, v, W_out):
    from concourse import bass_utils

    qt_full, encT, enc, wst, wht, v_pre, wot, lens = _prep_inputs(
        query, encoder_outputs, src_lengths, W_s, W_h, v, W_out
    )
    nc = _build(lens)

    in_maps = []
    for c in range(NCORES):
        qt_c = np.ascontiguousarray(
            qt_full[:, :, :, c * TC : (c + 1) * TC]
        )
        in_maps.append(
            {
                "qt": qt_c,
                "encT": encT,
                "enc": enc,
                "wst": wst,
                "wht": wht,
                "v": v_pre,
                "wot": wot,
            }
        )

    res = bass_utils.run_bass_kernel_spmd(nc, in_maps, core_ids=list(range(NCORES)))

    out = np.empty((B, T, H), np.float32)
    for c in range(NCORES):
        out[:, c * TC : (c + 1) * TC, :] = res.results[c]["out"]
    return out
